# revision 21
# baseline (speedup 1.0000x reference)
"""DarcyFlow operator Ax = conv2x2(4ch a-weighted shifts of x) zero-padded.

Strategy (8 NeuronCores, data-parallel over image ROWS):
  - Core c owns output rows [128c .. 128c+127] of all 16 images. The
    replicated coefficient field `a` stays tiny per core and its derived
    tiles are loaded once per core and reused across all 16 images.
  - The operator is decomposed into 4 elementwise products
        Q4 = a[r]   * x[r],   Q3 = a[r]   * x[r, +1col]
        Q1 = a[r-1] * x[r],   Q2 = a[r-1] * x[r, +1col]
    computed on the Vector engine, followed by the 16 conv taps
    accumulated on the Tensor engine as 8 banded matmuls into PSUM (row
    shifts live in the banded stationary matrix, column shifts in the
    moving-operand access pattern). ScalarE drains PSUM -> SBUF.
  - Everything runs in bf16 (fp32 PSUM accumulation): bf16 doubles DVE
    tensor_tensor throughput (2x_1P mode) and halves HBM traffic; the
    host upcasts the bf16 output back to fp32. To keep every DVE operand
    4-byte aligned (required for the 2x mode), the column-shifted
    products use a host-PREshifted copy of the a field (A0s/A1s) with
    the matmul moving-slice offset bumped by +1, instead of slicing x at
    an odd 2-byte offset.
  - Per image one 128-row window produces 126 output rows; the remaining
    2 rows/image are computed by one packed tail window (16 img x 4 rows).
  - Borders: output DRAM starts zeroed; stores skip border cols; the host
    drops the one garbage row computed at the global top/bottom edge.
"""

import numpy as np
import ml_dtypes

BF16 = ml_dtypes.bfloat16

B = 16
N = 1024
NCORES = 8
SLAB = N // NCORES  # 128
WX = N + 2          # padded x width (zero col both sides) = 1026
WQ = N + 1          # valid product width = 1025
WQE = 1026          # even-padded product width (4B-aligned bf16 chunks)
COLT = 512          # psum bank column tile

_K = np.array(
    [
        [[-1 / 6, 2 / 3], [-1 / 3, -1 / 6]],  # K1 (ch Q1)
        [[2 / 3, -1 / 6], [-1 / 6, -1 / 3]],  # K2 (ch Q2)
        [[-1 / 6, -1 / 3], [2 / 3, -1 / 6]],  # K3 (ch Q3)
        [[-1 / 3, -1 / 6], [-1 / 6, 2 / 3]],  # K4 (ch Q4)
    ],
    dtype=np.float32,
)

# pass order: (channel, dj). channel 0..3 <-> Q1,Q2,Q3,Q4
PASS_DEFS = [(ch, dj) for dj in (0, 1) for ch in (0, 1, 2, 3)]
# channel -> (which q buffer, column-chunk offset, extra col shift)
# q41 = [A0*X | A1*X], q32p = [A0s*X | A1s*X] (pre-shifted a)
CH_SLICE = [(0, WQE, 0), (1, WQE, 1), (1, 0, 1), (0, 0, 0)]


def _build_weights():
    """Host-built banded lhsT matrices for the 8 main + 8 tail passes."""
    wm = np.zeros((8, SLAB, SLAB), dtype=np.float32)
    wt = np.zeros((8, 64, 32), dtype=np.float32)
    for p, (ch, dj) in enumerate(PASS_DEFS):
        off = 0 if ch < 2 else -1  # Q1/Q2 band k-m in {0,1}; Q3/Q4 in {-1,0}
        for m in range(1, SLAB - 1):
            for di in range(2):
                wm[p, m + off + di, m] = _K[ch, di, dj]
        for b in range(16):
            for u in range(2):
                for di in range(2):
                    t = u + di + (1 if ch < 2 else 0)
                    wt[p, 4 * b + t, 2 * b + u] = _K[ch, di, dj]
    return (
        np.ascontiguousarray(wm.transpose(1, 0, 2).reshape(SLAB, 8 * SLAB)).astype(BF16),
        np.ascontiguousarray(wt.transpose(1, 0, 2).reshape(64, 8 * 32)).astype(BF16),
    )


def _shard_inputs(x, a):
    """Per-core padded bf16 input arrays. x: [B,1,N,N], a: [1,1,N-1,N-1]."""
    x = np.asarray(x, dtype=np.float32).reshape(B, N, N)
    a = np.asarray(a, dtype=np.float32).reshape(N - 1, N - 1)

    # zero-padded a lookup: arow(r) valid for r in [0, N-2]
    apad = np.zeros((N + 2, WQ), dtype=np.float32)
    apad[1:N, 1:N] = a  # apad[r+1, 1:N] = a[r]

    def afield(rows):
        """[len(rows), 2*WQE] = [A | As]: a rows plus the same rows
        column-shifted right by one (zero col 0)."""
        m = np.zeros((len(rows), 2 * WQE), dtype=np.float32)
        for k, r in enumerate(rows):
            m[k, 0:WQ] = apad[r + 1]
            m[k, WQE + 1 : WQE + 1 + WQ] = apad[r + 1]
        return m

    wm, wt = _build_weights()
    shards = []
    for c in range(NCORES):
        r0 = c * SLAB
        xc = np.zeros((B, SLAB + 2, WX), dtype=np.float32)
        lo = max(0, r0 - 1)
        hi = min(N, r0 + SLAB + 1)
        xc[:, lo - (r0 - 1) : hi - (r0 - 1), 1 : N + 1] = x[:, lo:hi, :]
        xc = xc.astype(BF16)

        rows_m0 = [r0 - 1 + k for k in range(SLAB)]
        rows_m1 = [r0 - 2 + k for k in range(SLAB)]
        rows_t0 = [r0 + 125 + t for _ in range(16) for t in range(4)]
        rows_t1 = [r0 + 124 + t for _ in range(16) for t in range(4)]
        f0, f1 = afield(rows_m0), afield(rows_m1)
        g0, g1 = afield(rows_t0), afield(rows_t1)
        # layout [A0 | A1 | A0s | A1s], each chunk WQE wide
        a01m = np.hstack(
            [f0[:, :WQE], f1[:, :WQE], f0[:, WQE:], f1[:, WQE:]]
        ).astype(BF16)
        a01t = np.hstack(
            [g0[:, :WQE], g1[:, :WQE], g0[:, WQE:], g1[:, WQE:]]
        ).astype(BF16)
        shards.append(
            {
                "xc": xc,
                "xt": np.ascontiguousarray(
                    xc[:, SLAB - 2 : SLAB + 2, :].reshape(64, WX)
                ),
                "a01m": np.ascontiguousarray(a01m),
                "a01t": np.ascontiguousarray(a01t),
                "wm": wm,
                "wt": wt,
            }
        )
    return shards


_CACHE = {}


def _build_module(iters=1, variant="full"):
    """Build + compile the (identical-program) per-core Bass module.

    iters > 1 wraps the compute in a hardware For loop (benchmarking).
    variant: "full" | "dma" (loads only) | "dve" (loads+products) |
             "nodve" (loads+matmuls+stores, skip products) — timing probes.
    """
    key = ("nc", iters, variant)
    if key in _CACHE:
        return _CACHE[key]

    import concourse.bacc as bacc
    import concourse.tile as tile
    from concourse import mybir

    bf16 = mybir.dt.bfloat16

    nc = bacc.Bacc("TRN2", target_bir_lowering=False, debug=False,
                   num_devices=NCORES)

    xc_d = nc.dram_tensor("xc", [B, SLAB + 2, WX], bf16, kind="ExternalInput").ap()
    xt_d = nc.dram_tensor("xt", [64, WX], bf16, kind="ExternalInput").ap()
    a01m_d = nc.dram_tensor("a01m", [SLAB, 4 * WQE], bf16, kind="ExternalInput").ap()
    a01t_d = nc.dram_tensor("a01t", [64, 4 * WQE], bf16, kind="ExternalInput").ap()
    wm_d = nc.dram_tensor("wm", [SLAB, 8 * SLAB], bf16, kind="ExternalInput").ap()
    wt_d = nc.dram_tensor("wt", [64, 8 * 32], bf16, kind="ExternalInput").ap()
    out_d = nc.dram_tensor("out", [B, SLAB, N], bf16, kind="ExternalOutput").ap()
    outt_d = nc.dram_tensor("outt", [32, N], bf16, kind="ExternalOutput").ap()

    with tile.TileContext(nc) as tc:
        with (
            tc.tile_pool(name="const", bufs=1) as const,
            tc.tile_pool(name="xin", bufs=4) as xin,
            tc.tile_pool(name="prod", bufs=3) as prod,
            tc.tile_pool(name="stage", bufs=4) as stage,
            tc.tile_pool(name="psum", bufs=6, space="PSUM") as psum,
        ):
            # window-0-gating constants first (a01t/wt only gate the tail).
            # a01m is split across two DGE queues so the q41 half lands
            # (and window 0 can start) in half the time.
            A01m = const.tile([SLAB, 4 * WQE], bf16)
            nc.gpsimd.dma_start(A01m[:], a01m_d[:])
            Wm = const.tile([SLAB, 8 * SLAB], bf16)
            nc.scalar.dma_start(Wm[:], wm_d[:])
            A01t = const.tile([64, 4 * WQE], bf16)
            nc.gpsimd.dma_start(A01t[:], a01t_d[:])
            Wt = const.tile([64, 8 * 32], bf16)
            nc.scalar.dma_start(Wt[:], wt_d[:])

            # PE warmup: dummy matmuls on a scratch tile bridge the
            # DMA-bound startup window so the p-state / HAM ramp is paid
            # before the real matmul stream begins. They write the tail's
            # scratch PSUM bank, which is never read here.
            if variant in ("full", "nodve"):
                scratch = const.tile([SLAB, 64], bf16)
                nc.vector.memset(scratch[:, 0:2], 0.0)
                rhs = scratch[:, 0:2][:, None, :].broadcast_to([SLAB, 256, 2])
                for _ in range(24):
                    wps = psum.tile([32, COLT], mybir.dt.float32,
                                    name="ps_64", tag="ps_64", bufs=2)
                    nc.tensor.matmul(wps[:], scratch[:, 0:32], rhs,
                                     start=True, stop=True)

            def window(X, A01, P, M, wtile, wstride, st, ps_bufs, store=None):
                """One banded-stencil window.
                X: [P, WX] input tile, A01: [P, 4*WQE] = [A0|A1|A0s|A1s],
                M: out partitions, wtile: weights, st: staging tile.
                """
                if variant == "dma":
                    return
                q = [None, None]  # q[0] = q41, q[1] = q32p
                for h in range(2):
                    q[h] = prod.tile([P, 2 * WQE], bf16,
                                     name=f"q{h}_{P}", tag=f"q{h}_{P}")
                    if variant == "nodve":
                        nc.vector.tensor_scalar_mul(
                            q[h][:, 0:2], X[:, 0:2], 1.0)
                    if variant in ("full", "dve"):
                        nc.vector.tensor_mul(
                            q[h][:].rearrange("p (c w) -> p c w", c=2),
                            A01[:, 2 * h * WQE : 2 * (h + 1) * WQE].rearrange(
                                "p (c w) -> p c w", c=2),
                            X[:, 0:WQE][:, None, :].broadcast_to([P, 2, WQE]),
                        )
                if variant in ("dma", "dve"):
                    return
                for t in range(2):
                    ps = psum.tile([M, COLT], mybir.dt.float32,
                                   name=f"ps_{P}", tag=f"ps_{P}", bufs=ps_bufs)
                    for p, (ch, dj) in enumerate(PASS_DEFS):
                        hbuf, off, sh = CH_SLICE[ch]
                        base = off + t * COLT + dj + sh
                        nc.tensor.matmul(
                            ps[:],
                            wtile[:, p * wstride : (p + 1) * wstride],
                            q[hbuf][:, base : base + COLT],
                            start=(p == 0),
                            stop=(p == 7),
                        )
                    nc.scalar.copy(st[:, t * COLT : (t + 1) * COLT], ps[:])
                    if store is not None and variant in ("full", "nodve"):
                        store(t, st)

            # column range of tile t, clipped to the stored interior cols
            TCOL = [(1, COLT), (COLT, N - 1)]

            def body():
                # 16 main windows (one per image)
                for b in range(B):
                    X = xin.tile([SLAB, WX], bf16, name="xw", tag="xw")
                    nc.sync.dma_start(X[:], xc_d[b, 0:SLAB, :])
                    st = stage.tile([SLAB, N], bf16, name="stm", tag="stm")

                    def store_m(t, st, b=b):
                        if t == 1:
                            nc.sync.dma_start(out_d[b, 0 : SLAB - 2, 1 : N - 1],
                                              st[1 : SLAB - 1, 1 : N - 1])

                    window(X, A01m, SLAB, SLAB, Wm, SLAB, st, 6, store_m)

                # packed tail: 16 images x rows 126..129 -> out rows 126,127
                Xt = xin.tile([64, WX], bf16, name="xtw", tag="xtw")
                nc.sync.dma_start(Xt[:], xt_d[:])
                stt = stage.tile([32, N], bf16, name="stt", tag="stt")

                def store_t(t, stt):
                    c0, c1 = TCOL[t]
                    nc.sync.dma_start(outt_d[:, c0:c1], stt[:, c0:c1])

                window(Xt, A01t, 64, 32, Wt, 32, stt, 2, store_t)

            if iters == 1:
                body()
            else:
                with tc.For_i(0, iters, 1):
                    body()

    nc.compile()
    _CACHE[key] = nc
    return nc


def run(inputs, trace=False, trace_kwargs=None, iters=1, variant="full"):
    """Run the sharded kernel; returns (full_output, BassKernelResults)."""
    from concourse.bass_utils import run_bass_kernel_spmd

    nc = _build_module(iters, variant)
    in_maps = _shard_inputs(inputs["x"], inputs["a"])
    res = run_bass_kernel_spmd(
        nc,
        in_maps,
        core_ids=list(range(NCORES)),
        trace=trace,
        **(trace_kwargs or {}),
    )
    full = np.zeros((B, 1, N, N), dtype=np.float32)
    for c in range(NCORES):
        oc = np.array(res.results[c]["out"]).astype(np.float32)  # [B, SLAB, N]
        oc[:, SLAB - 2 : SLAB, :] = (
            np.array(res.results[c]["outt"]).astype(np.float32).reshape(B, 2, N)
        )
        r0 = c * SLAB
        lo = 1 if c == 0 else 0            # drop garbage global row 0
        hi = SLAB - 1 if c == NCORES - 1 else SLAB  # drop garbage row N-1
        full[:, 0, r0 + lo : r0 + hi, 1 : N - 1] = oc[:, lo:hi, 1 : N - 1]
    return full, res


def kernel(**inputs) -> np.ndarray:
    out, _ = run(inputs, trace=False)
    return out
